# revision 24
# baseline (speedup 1.0000x reference)
"""Trainium2 Bass kernel for nn_ChannelMerger (v21).

Computation (per batch b):
    emb   = fourier_emb(positions[b])            # [C, D]   D=288  (host)
    scores= emb @ heads.T                        # [C, O]   O=270  (device)
    w     = softmax(scores + mask_offset, axis=C)           (device exp/sums)
    out[b]= (w.T @ meg[b])                       # [O, T]   (device)

Sharding: data-parallel over batch B=32 across 8 cores (4 batches/core).

Design (trace-driven):
  - fourier embedding computed on host; embT uploaded bf16.  Removes
    the Sin activation so the kernel needs ONE activation table set
    (exp_and_others: Exp + Copy), pre-triggered at t=0 by a dummy Exp.
  - C contraction chunks 96/96/96 (rows 177..191 duplicated, masked
    to zero weight): UNIFORM stationary shapes keep the LDWEIGHTS
    background-buffer pull-ahead working — mixing 128- and 17-row
    stationaries costs ~+100 ns on every shape-change edge (~2x
    stream slowdown).
  - all four batches' weights (scores+Exp) run up-front: with cst2
    arriving ~11us on the sync ring this is real PE work that spans
    the load lead-in (the psc pool intentionally paces scores to the
    ACT Exp chain), keeping the PE HAM-warm until big(0)'s data lands.
  - dma_start TRIGGER instructions execute on the issuing engine's
    queue and cost 0.6-1us each, so the scalar (ACT) engine — which
    must run Exp and half the psum evacuations at a 244ns-slack
    cadence — issues NO DMAs at all.  Loads ride the sync (HWDGE) and
    gpsimd (SWDGE) rings; stores ride sync too, enqueued after its
    loads (which drain by the time the first store is ready).
  - a dummy-matmul warm-up burst flips the PE HAM clock gate (K=8/8)
    during the otherwise-dead DMA lead-in.
  - the big-matmul stream runs fully DENSE and is gated on meg[1]'s
    first half (a 2-column matmul touching both pieces): full pace
    demands ~390 GB/s of HBM (meg in + stores out) against ~270
    available, so an early start would starve mid-stream — and any
    >3.4us PE stall re-throttles the HAM clock gate to 1.2 GHz, a
    self-sustaining ~3x collapse.  Ring-FIFO deadline order then
    guarantees b2/b3 arrive >=5us before consumption.
  - a 9-matmul burst after the gate re-warms the PE (the gate wait
    itself re-throttles it).
  - sume (softmax denominator) matmuls run together after big(0), so
    the ACT Exp chain never stalls the PE FIFO and the uniform
    stationary-shape stream breaks only twice; 1/sum on host.
  - ALL output stores on the (otherwise untouched) sync ring.

Output dram layout is [BPC, 128, 32*270] bf16 with out[b, t, o] at
[b, t % 128, (t // 128)*270 + o]; host untangles, upcasts, divides by
the softmax sums.
"""

import math

import numpy as np

import concourse.bacc as bacc
import concourse.bass as bass
import concourse.mybir as mybir
from concourse.bass_utils import run_bass_kernel_spmd
from concourse.tile import TileContext

# Problem shape (hardcoded per contract)
B, C, T = 32, 273, 4096
O, D = 270, 288
NF = 12            # fourier freqs per axis (sqrt(D/2))
MARGIN = 0.1
NCORES = 8
BPC = B // NCORES  # batches per core

C0S = [0, 96, 177]           # C contraction chunk starts (chunk 2 overlaps)
CWS = [96, 96, 96]           # C chunk widths; rows 177..191 masked in chunk 2
NDUP = 2 * 96 - 177          # 15 duplicated rows at the head of chunk 2
DK = 96                      # D chunk width (3 chunks of 96 = 288)
CP = 274                     # C padded to even for the embT layout

TCH = 128          # T chunk = psum partition dim of the big matmul
NTH = T // TCH     # 32
OW = NTH * O       # out staging columns per partition (8640)

NEG_BIG = -1.0e30  # stands in for -inf on masked channels

# cst1 ([96, CW1] bf16): headsT chunks, then embT(b=0)
HD_C0 = 0
EMB0_C0 = 3 * O
CW1 = EMB0_C0 + 3 * CP
# cst2 ([96, CW2] bf16): embT(b=1..3)
CW2 = 3 * 3 * CP

F32 = mybir.dt.float32
BF16 = mybir.dt.bfloat16

_CACHE = {}
LAST_RESULTS = None         # BassKernelResults of the most recent run (for test.py)


def _host_emb(positions):
    """fourier_emb on host: positions [B, C, 2] f32 -> emb [B, C, D] f64."""
    p = (2.0 * math.pi / (1.0 + 2.0 * MARGIN)) * np.arange(NF, dtype=np.float64)
    pos = positions.astype(np.float64) + MARGIN
    loc = (
        pos[..., 0, None, None] * p[:, None] + pos[..., 1, None, None] * p[None, :]
    ).reshape(*positions.shape[:-1], NF * NF)
    return np.concatenate([np.cos(loc), np.sin(loc)], axis=-1)


def _build_program():
    nc = bacc.Bacc(
        trn_type="TRN2",
        target_bir_lowering=False,
        debug=False,
        dynamic_dma_scratch_size=32768,
    )

    # meg pre-chunked on host: megC01[b, p, j, t] = meg[b, j*96 + p, t],
    # megC2[b, p, t] = meg[b, 177 + p, t].
    megC01 = nc.dram_tensor("megC01", [BPC, 96, 2, T], BF16, kind="ExternalInput").ap()
    megC2 = nc.dram_tensor("megC2", [BPC, 96, T], BF16, kind="ExternalInput").ap()
    cst1d = nc.dram_tensor("cst1d", [96, CW1], BF16, kind="ExternalInput").ap()
    cst2d = nc.dram_tensor("cst2d", [96, CW2], BF16, kind="ExternalInput").ap()
    cstOd = nc.dram_tensor("cstOd", [96, 3 * BPC], F32, kind="ExternalInput").ap()
    out = nc.dram_tensor("out", [BPC, TCH, OW], BF16, kind="ExternalOutput").ap()
    sumd = nc.dram_tensor("sumd", [1, BPC * O], F32, kind="ExternalOutput").ap()

    with TileContext(nc) as tc:
        with (
            tc.tile_pool(name="singles", bufs=1) as singles,
            tc.tile_pool(name="w", bufs=4) as wp,
            tc.tile_pool(name="megp", bufs=4) as megp,
            tc.tile_pool(name="outp", bufs=4) as outp,
            tc.tile_pool(name="psc", bufs=2, space="PSUM") as psc,
            tc.tile_pool(name="psbig", bufs=6, space="PSUM") as psbig,
        ):
            # ---- dummy Exp first: pulls the exp_and_others ACT table load
            # to t=0 (it is the only table set the kernel ever needs)
            ones_sb = singles.tile([128, 1], BF16, name="ones_sb")
            nc.vector.memset(ones_sb, 1.0)
            scratch = singles.tile([1, 1], F32, name="scratch")
            nc.scalar.activation(
                scratch, ones_sb[0:1, 0:1], mybir.ActivationFunctionType.Exp
            )

            # ---- HAM warm-up: dense dummy matmul burst on scratch data ----
            warm_sb = singles.tile([128, 512], BF16, name="warm_sb")
            nc.vector.memset(warm_sb, 0.0)
            warm_ps = psc.tile([128, 512], F32, name="warm_ps", tag="sc")
            for i in range(10):
                nc.tensor.matmul(
                    warm_ps,
                    warm_sb[:, 0:128],
                    warm_sb,
                    start=(i == 0),
                    stop=(i == 9),
                )

            # ---- const loads: cst1 first on the SWDGE queue (fast early),
            # csts for later batches lead the sync ring
            cst1 = singles.tile([96, CW1], BF16, name="cst1")
            nc.gpsimd.dma_start(out=cst1, in_=cst1d)
            cstO = singles.tile([96, 3 * BPC], F32, name="cstO")
            nc.sync.dma_start(out=cstO, in_=cstOd)
            cst2 = singles.tile([96, CW2], BF16, name="cst2")
            nc.sync.dma_start(out=cst2, in_=cst2d)

            headsT = [
                cst1[:, HD_C0 + k * O : HD_C0 + (k + 1) * O] for k in range(3)
            ]

            # ---- meg loads: mB whole, mA in four 1024-column pieces,
            # even pieces on gpsimd, odd on sync, in deadline order
            megs = {}
            PQ = T // 4
            for b in range(BPC):
                mA = megp.tile([96, 2, T], BF16, name=f"megA_b{b}", tag="megA")
                mB = megp.tile([96, T], BF16, name=f"megB_b{b}", tag="megB")
                nc.gpsimd.dma_start(out=mB, in_=megC2[b])
                for p in range(4):
                    eng = nc.gpsimd if p % 2 == 0 else nc.sync
                    eng.dma_start(
                        out=mA[:, :, p * PQ : (p + 1) * PQ],
                        in_=megC01[b, :, :, p * PQ : (p + 1) * PQ],
                    )
                megs[b] = (mA, mB)

            def embT(b, k):
                if b == 0:
                    return cst1[:, EMB0_C0 + k * CP : EMB0_C0 + k * CP + CP]
                i = (b - 1) * 3 + k
                return cst2[:, i * CP : i * CP + CP]

            sume_sb = singles.tile([1, BPC * O], F32, name="sume_sb")

            expT = {}

            # ---- weights: scores matmuls + Exp for one batch ----
            def weights(b):
                for j in range(3):
                    c0, cw = C0S[j], CWS[j]
                    sc = psc.tile([128, O], F32, name=f"sc_b{b}j{j}", tag="sc")
                    for k in range(3):
                        nc.tensor.matmul(
                            sc[0:cw, :],
                            embT(b, k)[:, c0 : c0 + cw],
                            headsT[k],
                            start=(k == 0),
                            stop=(k == 2),
                        )
                    ex = wp.tile([128, O], BF16, name=f"expT_b{b}j{j}", tag=f"expT{j}")
                    nc.scalar.activation(
                        ex[0:cw, :],
                        sc[0:cw, :],
                        mybir.ActivationFunctionType.Exp,
                        bias=cstO[0:cw, b * 3 + j : b * 3 + j + 1],
                    )
                    expT[(b, j)] = ex

            def w_sume(b):
                sume = psc.tile([1, O], F32, name=f"sume_b{b}", tag="sc")
                for j in range(3):
                    cw = CWS[j]
                    nc.tensor.matmul(
                        sume,
                        ones_sb[0:cw, :],
                        expT[(b, j)][0:cw, :],
                        start=(j == 0),
                        stop=(j == 2),
                    )
                nc.vector.tensor_copy(out=sume_sb[:, b * O : (b + 1) * O], in_=sume)

            # ---- big matmuls ----
            def big_matmul(b):
                mA, mB = megs[b]
                ob = outp.tile([TCH, OW], BF16, name=f"out_b{b}", tag="out")
                nparts = 2 if b + 1 < BPC else 8
                step = OW // nparts
                for th in range(NTH):
                    pb = psbig.tile([TCH, O], F32, name=f"pb_b{b}t{th}", tag="pb")
                    for j in range(3):
                        cw = CWS[j]
                        if j < 2:
                            lhsT = mA[:, j, th * TCH : (th + 1) * TCH]
                        else:
                            lhsT = mB[:, th * TCH : (th + 1) * TCH]
                        nc.tensor.matmul(
                            pb,
                            lhsT,
                            expT[(b, j)][0:cw, :],
                            start=(j == 0),
                            stop=(j == 2),
                        )
                    dst = ob[:, th * O : (th + 1) * O]
                    if th % 2 == 0:
                        nc.vector.tensor_copy(out=dst, in_=pb)
                    else:
                        nc.scalar.activation(
                            dst, pb, mybir.ActivationFunctionType.Copy
                        )
                    done = (th + 1) * O
                    if done % step == 0:
                        q = done // step - 1
                        nc.sync.dma_start(
                            out=out[b, :, q * step : (q + 1) * step],
                            in_=ob[:, q * step : (q + 1) * step],
                        )

            for b in range(BPC):
                weights(b)
            # ---- filler + stream gate + re-warm: the no-dep filler
            # burst spans the idle window between the last weights work
            # and the gate firing, so the PE HAM clock gate stays warm;
            # the gate matmul touches one column of each of meg[1]'s
            # first two pieces, so the dense stream only starts once
            # loads are ~half done; the short burst after it re-warms
            # the PE if a slow-load run idled through the filler anyway.
            mA1g = megs[1][0]
            gate_ps = psc.tile([128, 512], F32, name="gate_ps", tag="sc")
            for i in range(36):
                nc.tensor.matmul(
                    gate_ps[:, 0:256], warm_sb[:, 0:128], warm_sb[:, 0:256],
                    start=(i == 0), stop=(i == 35),
                )
            nc.tensor.matmul(
                gate_ps[:, 0:2], warm_sb[0:96, 0:128],
                mA1g[:, 0, 0 : 2 * PQ : PQ], start=True, stop=True,
            )
            for i in range(9):
                nc.tensor.matmul(
                    gate_ps, warm_sb[:, 0:128], warm_sb,
                    start=(i == 0), stop=(i == 8),
                )
            big_matmul(0)
            for b in range(BPC):
                w_sume(b)
            nc.gpsimd.dma_start(out=sumd, in_=sume_sb)
            big_matmul(1)
            big_matmul(2)
            big_matmul(3)
    nc.compile()
    return nc


def _get_program():
    if "nc" not in _CACHE:
        _CACHE["nc"] = _build_program()
    return _CACHE["nc"]


def kernel(meg, positions, heads, invalid_mask, trace=False):
    global LAST_RESULTS
    bf16 = mybir.dt.np(BF16)
    meg = np.asarray(meg, dtype=np.float32)
    positions = np.asarray(positions, dtype=np.float32)
    heads = np.asarray(heads, dtype=np.float32)

    megb = meg.astype(bf16)                                      # [B, C, T] bf16
    megC01 = np.ascontiguousarray(
        megb[:, 0:192, :].reshape(B, 2, 96, T).transpose(0, 2, 1, 3)
    )                                                            # [B, 96, 2, T]
    megC2 = np.ascontiguousarray(megb[:, 177:273, :])            # [B, 96, T]

    emb = _host_emb(positions)                                   # [B, C, D] f64
    headsT = heads.T                                             # [D, O]

    cst1 = np.zeros((NCORES, 96, CW1), bf16)
    cst2 = np.zeros((NCORES, 96, CW2), bf16)
    cstO = np.zeros((NCORES, 96, 3 * BPC), np.float32)
    for k in range(3):
        cst1[:, :, HD_C0 + k * O : HD_C0 + (k + 1) * O] = headsT[
            k * DK : (k + 1) * DK, :
        ].astype(bf16)

    maskf = np.asarray(invalid_mask, dtype=bool)                 # [B, C]
    for cix in range(NCORES):
        for bl in range(BPC):
            bg = cix * BPC + bl
            # embT(b,k)[d, c] = emb[bg, c, k*96 + d]
            eT = emb[bg].T.astype(bf16)                          # [D, C]
            for k in range(3):
                blk = eT[k * DK : (k + 1) * DK, :]               # [96, C]
                if bl == 0:
                    cst1[cix, :, EMB0_C0 + k * CP : EMB0_C0 + k * CP + C] = blk
                else:
                    i = (bl - 1) * 3 + k
                    cst2[cix, :, i * CP : i * CP + C] = blk
            for j in range(3):
                c0, cw = C0S[j], CWS[j]
                m = maskf[bg, c0 : c0 + cw].astype(np.float32) * NEG_BIG
                if j == 2:
                    m[:NDUP] = NEG_BIG
                cstO[cix, 0:cw, bl * 3 + j] = m

    nc = _get_program()
    in_maps = []
    for cix in range(NCORES):
        s = slice(cix * BPC, (cix + 1) * BPC)
        in_maps.append(
            {
                "megC01": np.ascontiguousarray(megC01[s]),
                "megC2": np.ascontiguousarray(megC2[s]),
                "cst1d": np.ascontiguousarray(cst1[cix]),
                "cst2d": np.ascontiguousarray(cst2[cix]),
                "cstOd": np.ascontiguousarray(cstO[cix]),
            }
        )

    res = run_bass_kernel_spmd(nc, in_maps, core_ids=list(range(NCORES)), trace=trace)
    LAST_RESULTS = res
    # out[b, t, o] lives at [b, t % 128, (t // 128)*270 + o], unnormalized
    raw = np.concatenate([r["out"] for r in res.results], axis=0)  # [B,128,OW]
    sume = np.concatenate(
        [r["sumd"].reshape(BPC, O) for r in res.results], axis=0
    )  # [B, O]
    full = raw.astype(np.float32).reshape(B, TCH, NTH, O) / sume[:, None, None, :]
    return np.ascontiguousarray(full.transpose(0, 3, 2, 1).reshape(B, O, T))
